# revision 1
# baseline (speedup 1.0000x reference)
"""Causal multi-head attention block (qkv_proj -> causal softmax attention ->
out_proj) distributed over 8 NeuronCores.

Sharding: batch x head. B=2, H=8 -> 16 (b,h) units; core i handles batch
i//4 and heads {2*(i%4), 2*(i%4)+1}. Each core computes q/k/v for its two
heads, causal flash-style attention (no max subtraction -- scores are
bounded ~3), and the out_proj partial for its heads' columns. Host sums the
4 partials per batch and adds out_b.

Device layouts (host pre-transposes weights/x -- pure layout, no FLOPs):
  xT     [512, 4096]  x[b].T
  wqkvT  [512, 384]   qkv weight columns for the 2 heads, c-major
  bqkv   [128, 3]     q/k/v bias per f-row ([h0 d64 | h1 d64])
  woT    [128, 512]   out_w columns for the 2 heads, transposed
Output per core: out_part [4096, 512] = sum over its 2 heads of
  (softmax(q k^T / 8) v) @ out_w_head.T  (rows normalized on device).

All matmuls run float32r (fp22 multiplies, fp32 accumulate): ~1e-4 rel err.
"""

import numpy as np

import concourse.bass as bass
from concourse import bacc
import concourse.mybir as mybir
import concourse.tile as tile
from concourse.bass_utils import run_bass_kernel_spmd

F32 = mybir.dt.float32
F32R = mybir.dt.float32r

B, N, C = 2, 4096, 512
H, D = 8, 64
NCORES = 8
HPC = 2               # heads per core
NT = N // 512         # 8 tiles of 512 (qkv n-tiles and attention q-tiles)
CC = C // 128         # 4 contraction chunks for the projections
KC = N // 128         # 32 k-chunks
G = 2                 # k-chunks per exp group ([128,1024] psum -> one ACT call)


def build_nc(loop_n=None):
    nc = bacc.Bacc()

    xT = nc.declare_dram_parameter("xT", [C, N], F32R, isOutput=False)
    wqkvT = nc.declare_dram_parameter("wqkvT", [C, 3 * 128], F32R, isOutput=False)
    bqkv = nc.declare_dram_parameter("bqkv", [128, 3], F32, isOutput=False)
    woT = nc.declare_dram_parameter("woT", [64, 2, 512], F32R, isOutput=False)
    out_part = nc.declare_dram_parameter("out_part", [N, C], F32, isOutput=True)
    # scratch to reshape softmax row-sums across partitions ([1,512] -> [4,128])
    sums_scratch = nc.dram_tensor("sums_scratch", [HPC, NT, 512], F32R)

    with tile.TileContext(nc) as tc:
        with (
            tc.tile_pool(name="singles", bufs=1) as singles,
            tc.tile_pool(name="xin", bufs=4) as xin_pool,
            tc.tile_pool(name="vtmp", bufs=3) as vtmp_pool,
            tc.tile_pool(name="expp", bufs=6) as exp_pool,
            tc.tile_pool(name="tout", bufs=4) as t_pool,
            tc.tile_pool(name="small", bufs=8) as small_pool,
            tc.tile_pool(name="fin", bufs=4) as fin_pool,
            tc.tile_pool(name="ps1", bufs=2, space="PSUM") as ps1,
            tc.tile_pool(name="psav", bufs=2, space="PSUM") as psav,
            tc.tile_pool(name="ps2", bufs=2, space="PSUM") as ps2,
        ):
            # ---- persistent tiles ----
            w_sb = singles.tile([128, CC, 3, 128], F32R)       # wqkvT chunks
            wo_sb = singles.tile([64, 2, 512], F32R)
            bqkv_sb = singles.tile([128, 3], F32)
            ident_f = singles.tile([128, 128], F32)
            ident = singles.tile([128, 128], F32R)
            ones_f = singles.tile([128, 64], F32)
            qT_sb = singles.tile([128, N], F32R)               # [h0 d|h1 d] x n
            kT_sb = singles.tile([128, N], F32R)
            v_sb = singles.tile([128, KC, 2, 65], F32R)        # per head [v|1]

            nc.gpsimd.dma_start(
                out=w_sb,
                in_=wqkvT[:].rearrange("(cc p) (pr f) -> p cc pr f", p=128, f=128),
            )
            nc.gpsimd.dma_start(out=wo_sb, in_=woT[:])
            nc.gpsimd.dma_start(out=bqkv_sb, in_=bqkv[:])
            from concourse.masks import make_identity

            make_identity(nc, ident_f)
            nc.vector.tensor_copy(ident, ident_f)
            nc.gpsimd.memset(ones_f, 1.0)
            nc.vector.tensor_copy(
                v_sb[:, :, :, 64:65],
                ones_f[:, 0:64].rearrange("p (a b c) -> p a b c", b=2, c=1),
            )

            # ---- fused pipeline: project tile t, then attend q-tile t ----
            import contextlib
            loop_cm = tc.For_i(0, loop_n, 1) if loop_n else contextlib.nullcontext()
            with loop_cm:
              def emit_outproj(jo, r_jo):
                  for c in range(4):
                      i = 4 * jo + c
                      po = []
                      for h in range(HPC):
                          t_sb, _ = r_jo[h]
                          pso = ps1.tile([128, 512], F32, tag="ps1")
                          nc.tensor.matmul(
                              pso,
                              t_sb[0:64, c * 128:(c + 1) * 128],
                              wo_sb[:, h, :],
                              start=True, stop=True,
                          )
                          po.append(pso)
                      fin = fin_pool.tile([128, 512], F32, tag="fin")
                      nc.vector.tensor_scalar(
                          out=fin, in0=po[0],
                          scalar1=r_jo[0][1][:, c:c + 1], scalar2=None,
                          op0=mybir.AluOpType.mult,
                      )
                      nc.vector.scalar_tensor_tensor(
                          out=fin, in0=po[1],
                          scalar=r_jo[1][1][:, c:c + 1], in1=fin,
                          op0=mybir.AluOpType.mult, op1=mybir.AluOpType.add,
                      )
                      nc.sync.dma_start(
                          out=out_part[i * 128:(i + 1) * 128, :], in_=fin
                      )

              def emit_tail(jo, tps):
                  # [out|sums] -> SBUF, sums -> [128,4] recip, out_proj
                  r_jo = []
                  for h in range(HPC):
                      t_sb = t_pool.tile([65, 512], F32R, tag="tout")
                      nc.vector.tensor_copy(t_sb, tps[h])
                      stage = small_pool.tile([4, 128], F32R, tag="stage")
                      for s in range(4):
                          nc.sync.dma_start(
                              out=stage[s:s + 1, :],
                              in_=t_sb[64:65, s * 128:(s + 1) * 128],
                          )
                      pss = ps1.tile([128, 4], F32R, tag="ps1")
                      nc.tensor.transpose(pss, stage, ident[0:4, 0:4])
                      sums_c = small_pool.tile([128, 4], F32, tag="sums")
                      nc.vector.tensor_copy(sums_c, pss)
                      r_c = small_pool.tile([128, 4], F32, tag="recip")
                      nc.vector.reciprocal_approx_fast(out=r_c, in_=sums_c)
                      r_jo.append((t_sb, r_c))
                  emit_outproj(jo, r_jo)

              prev_out = None
              for t in range(NT):
                  ns = slice(t * 512, (t + 1) * 512)
                  x_t = xin_pool.tile([128, CC, 512], F32R, tag="xin")
                  nc.sync.dma_start(
                      out=x_t,
                      in_=xT[:, ns].rearrange("(cc p) n -> p cc n", p=128),
                  )
                  for pr, dst in ((0, qT_sb), (1, kT_sb)):
                      ps = ps1.tile([128, 512], F32, tag="ps1")
                      for cc in range(CC):
                          nc.tensor.matmul(
                              ps, w_sb[:, cc, pr, :], x_t[:, cc, :],
                              start=(cc == 0), stop=(cc == CC - 1),
                          )
                      nc.vector.tensor_scalar(
                          out=dst[:, ns], in0=ps,
                          scalar1=bqkv_sb[:, pr:pr + 1], scalar2=None,
                          op0=mybir.AluOpType.add,
                      )
                  # v: same [f, n] matmul, add bias, then transpose to [n, f]
                  ps = ps1.tile([128, 512], F32, tag="ps1")
                  for cc in range(CC):
                      nc.tensor.matmul(
                          ps, w_sb[:, cc, 2, :], x_t[:, cc, :],
                          start=(cc == 0), stop=(cc == CC - 1),
                      )
                  vt = vtmp_pool.tile([128, 512], F32R, tag="vtmp")
                  nc.vector.tensor_scalar(
                      out=vt, in0=ps,
                      scalar1=bqkv_sb[:, 2:3], scalar2=None,
                      op0=mybir.AluOpType.add,
                  )
                  pst = ps1.tile([128, 4, 128], F32R, tag="ps1")
                  for s in range(4):
                      nc.tensor.transpose(
                          pst[:, s, :], vt[:, s * 128:(s + 1) * 128], ident)
                  nc.vector.tensor_copy(
                      v_sb[:, 4 * t:4 * t + 4, :, 0:64],
                      pst.rearrange("p s (h c) -> p s h c", c=64),
                  )

                  if prev_out is not None:
                      emit_tail(*prev_out)
                      prev_out = None

                  # ---- attention + out_proj for q-tile j = t ----
                  j = t
                  qs = slice(j * 512, (j + 1) * 512)
                  to_ps = [psav.tile([65, 512], F32, tag="psav", name=f"to_ps{_h}")
                           for _h in range(HPC)]
                  ngroups = (4 * (j + 1)) // G

                  def emit_av(ga, ets):
                      for h in range(HPC):
                          for cl in range(G):
                              kc = ga * G + cl
                              nc.tensor.matmul(
                                  to_ps[h],
                                  v_sb[:, kc, h, :],
                                  ets[h][:, cl, :],
                                  start=(kc == 0), stop=(kc == 4 * j + 3),
                              )

                  prev_av = None
                  for g in range(ngroups):
                      scs = []
                      for h in range(HPC):
                          hs = slice(h * 64, (h + 1) * 64)
                          sc = ps2.tile([128, G, 512], F32, tag="ps2")
                          for cl in range(G):
                              kc = g * G + cl
                              nc.tensor.matmul(
                                  sc[:, cl, :],
                                  kT_sb[hs, kc * 128:(kc + 1) * 128],
                                  qT_sb[hs, qs],
                                  start=True, stop=True,
                              )
                          scs.append(sc)
                      ets = []
                      for h in range(HPC):
                          et = exp_pool.tile([128, G, 512], F32R, tag="expp",
                                             name=f"et{h}")
                          nc.scalar.activation(
                              et, scs[h], mybir.ActivationFunctionType.Exp,
                              scale=0.125,
                          )
                          if (g + 1) * G > 4 * j:  # group touches the diagonal
                              nc.gpsimd.affine_select(
                                  out=et, in_=et,
                                  compare_op=mybir.AluOpType.is_ge,
                                  fill=0.0,
                                  base=512 * j - 128 * G * g,
                                  channel_multiplier=-1,
                                  pattern=[[-128, G], [1, 512]],
                              )
                          ets.append(et)
                      if prev_av is not None:
                          emit_av(*prev_av)
                      prev_av = (g, ets)
                  emit_av(*prev_av)
                  prev_out = (j, to_ps)
              emit_tail(*prev_out)
    nc.finalize()
    return nc


def make_in_maps(x, qkv_w, qkv_b, out_w):
    x = np.asarray(x, np.float32)
    qkv_w = np.asarray(qkv_w, np.float32)
    qkv_b = np.asarray(qkv_b, np.float32)
    out_w = np.asarray(out_w, np.float32)
    in_maps = []
    for core in range(NCORES):
        b = core // 4
        h0 = HPC * (core % 4)
        rows = np.r_[h0 * D:(h0 + 1) * D, (h0 + 1) * D:(h0 + 2) * D]
        wq = qkv_w[rows]
        wk = qkv_w[C + rows]
        wv = qkv_w[2 * C + rows]
        in_maps.append({
            "xT": np.ascontiguousarray(x[b].T),
            "wqkvT": np.ascontiguousarray(
                np.concatenate([wq, wk, wv], 0).T),
            "bqkv": np.ascontiguousarray(
                np.stack([qkv_b[rows], qkv_b[C + rows], qkv_b[2 * C + rows]], 1)),
            "woT": np.ascontiguousarray(np.stack(
                [out_w[:, h0 * D:(h0 + 1) * D].T,
                 out_w[:, (h0 + 1) * D:(h0 + 2) * D].T], 1)),
        })
    return in_maps


def combine(results, out_b):
    parts = [r["out_part"] for r in results]
    out = np.empty((B, N, C), np.float32)
    for b in range(B):
        out[b] = parts[4 * b] + parts[4 * b + 1] + parts[4 * b + 2] \
            + parts[4 * b + 3] + np.asarray(out_b, np.float32)
    return out


_NC = None


def kernel(x, mask, qkv_w, qkv_b, out_w, out_b, **run_kwargs):
    global _NC
    del mask  # causal tril by construction; applied analytically on device
    if _NC is None:
        _NC = build_nc()
    in_maps = make_in_maps(x, qkv_w, qkv_b, out_w)
    res = run_bass_kernel_spmd(_NC, in_maps, list(range(NCORES)), **run_kwargs)
    out = combine(res.results, out_b)
    kernel.last_results = res
    return out



# revision 3
# speedup vs baseline: 1.1444x; 1.1444x over previous
"""Causal MHA block (qkv_proj -> causal softmax attention -> out_proj)
distributed over 8 NeuronCores. v2: chunk-granular scores/exp pipeline.

Sharding: batch x head. B=2, H=8 -> 16 (b,h) units; core i handles batch
i//4 and heads {2*(i%4), 2*(i%4)+1}.

v2 structure (vs v1): scores for BOTH heads of a k-chunk go into one
2-bank PSUM tile ([128, 2, 512]), one exp per chunk covers both heads,
so two chunks fit in PSUM concurrently -> scores(kc+1) overlaps exp(kc)
and the steady state is ACT-bound. AV rides in PE slack. out_proj pairs
are row-tiled ((0,0)/(64,0)) via a combined [128,512] AV tile and a
[128,512] wo layout. qkv projection of tile t+1 is interleaved into the
attention chunk stream of q-tile t so PE never stalls ACT at tile
boundaries.

Device layouts (host pre-transposes -- pure layout, no FLOPs):
  xT     [512, 4096]  x[b].T
  wqkvT  [512, 384]   qkv weight columns for the 2 heads, c-major
  bqkv   [128, 3]     q/k/v bias per f-row ([h0 d64 | h1 d64])
  woT    [128, 512]   rows 0-63 = out_w cols of h0 (transposed), 64-127 = h1
Output per core: out_part [4096, 512]; host sums 4 partials per batch
and adds out_b. All matmuls float32r.
"""

import numpy as np

import concourse.bass as bass
from concourse import bacc
import concourse.mybir as mybir
import concourse.tile as tile
from concourse.bass_utils import run_bass_kernel_spmd

F32 = mybir.dt.float32
F32R = mybir.dt.float32r
FP8 = mybir.dt.float8e4

B, N, C = 2, 4096, 512
H, D = 8, 64
NCORES = 8
HPC = 2               # heads per core
NT = N // 512         # 8 tiles of 512
CC = C // 128         # 4 contraction chunks for the projections
KC = N // 128         # 32 k-chunks


def build_nc(loop_n=None):
    nc = bacc.Bacc()

    xT = nc.declare_dram_parameter("xT", [C, N], F32R, isOutput=False)
    wqkvT = nc.declare_dram_parameter("wqkvT", [C, 3 * 128], F32R, isOutput=False)
    bqkv = nc.declare_dram_parameter("bqkv", [128, 3], F32, isOutput=False)
    woT = nc.declare_dram_parameter("woT", [128, 512], F32R, isOutput=False)
    out_part = nc.declare_dram_parameter("out_part", [N, C], F32, isOutput=True)

    with tile.TileContext(nc) as tc:
        with (
            tc.tile_pool(name="singles", bufs=1) as singles,
            tc.tile_pool(name="xin", bufs=3) as xin_pool,
            tc.tile_pool(name="vtmp", bufs=2) as vtmp_pool,
            tc.tile_pool(name="expp", bufs=4) as exp_pool,
            tc.tile_pool(name="tout", bufs=2) as t_pool,
            tc.tile_pool(name="small", bufs=8) as small_pool,
            tc.tile_pool(name="fin", bufs=3) as fin_pool,
            tc.tile_pool(name="ps1", bufs=2, space="PSUM") as ps1,
            tc.tile_pool(name="psav", bufs=2, space="PSUM") as psav,
            tc.tile_pool(name="ps2", bufs=2, space="PSUM") as ps2,
        ):
            # ---- persistent tiles ----
            w_sb = singles.tile([128, CC, 3, 128], F32R)       # wqkvT chunks
            wo_sb = singles.tile([128, 512], F32R)             # [h0 | h1] rows
            bqkv_sb = singles.tile([128, 3], F32)
            ident_f = singles.tile([128, 128], F32)
            ident = singles.tile([128, 128], F32R)
            ones_f = singles.tile([128, 64], F32)
            qT_sb = singles.tile([128, N], F32R)               # [h0 d|h1 d] x n
            kT_sb = singles.tile([128, N], F32R)
            v_sb = singles.tile([128, KC // 2, 2, 2, 80], FP8)  # [pair, ko, h, v|1]
            v0_sb = singles.tile([128, 4, 2, 65], F32R)         # chunks 0-3, exact

            nc.gpsimd.dma_start(
                out=w_sb,
                in_=wqkvT[:].rearrange("(cc p) (pr f) -> p cc pr f", p=128, f=128),
            )
            nc.gpsimd.dma_start(out=wo_sb, in_=woT[:])
            nc.gpsimd.dma_start(out=bqkv_sb, in_=bqkv[:])
            from concourse.masks import make_identity

            make_identity(nc, ident_f)
            nc.vector.tensor_copy(ident, ident_f)
            nc.gpsimd.memset(ones_f, 1.0)
            nc.vector.tensor_copy(
                v_sb[:, :, :, :, 64:65],
                ones_f[:, 0:64].rearrange("p (a b c d) -> p a b c d",
                                          b=2, c=2, d=1),
            )
            nc.vector.tensor_copy(
                v0_sb[:, :, :, 64:65],
                ones_f[:, 0:8].rearrange("p (a b c) -> p a b c", b=2, c=1),
            )

            import contextlib
            loop_cm = tc.For_i(0, loop_n, 1) if loop_n else contextlib.nullcontext()
            with loop_cm:
              def emit_xdma(t):
                  ns = slice(t * 512, (t + 1) * 512)
                  x_t = xin_pool.tile([128, CC, 512], F32R, tag="xin")
                  nc.sync.dma_start(
                      out=x_t,
                      in_=xT[:, ns].rearrange("(cc p) n -> p cc n", p=128),
                  )
                  return x_t

              def proj_pieces(t, x_t):
                  """qkv projection of tile t as a list of closures (PE
                  filler pieces interleaved into the attention stream)."""
                  ns = slice(t * 512, (t + 1) * 512)

                  def qk(pr, dst):
                      def f():
                          ps = ps1.tile([128, 512], F32, tag="ps1")
                          for cc in range(CC):
                              nc.tensor.matmul(
                                  ps, w_sb[:, cc, pr, :], x_t[:, cc, :],
                                  start=(cc == 0), stop=(cc == CC - 1),
                              )
                          nc.vector.tensor_scalar(
                              out=dst[:, ns], in0=ps,
                              scalar1=bqkv_sb[:, pr:pr + 1], scalar2=None,
                              op0=mybir.AluOpType.add,
                          )
                      return f

                  def vproj():
                      ps = ps1.tile([128, 512], F32, tag="ps1")
                      for cc in range(CC):
                          nc.tensor.matmul(
                              ps, w_sb[:, cc, 2, :], x_t[:, cc, :],
                              start=(cc == 0), stop=(cc == CC - 1),
                          )
                      vt = vtmp_pool.tile([128, 512], F32R, tag="vtmp")
                      nc.vector.tensor_scalar(
                          out=vt, in0=ps,
                          scalar1=bqkv_sb[:, 2:3], scalar2=None,
                          op0=mybir.AluOpType.add,
                      )
                      pst = ps1.tile([128, 4, 128], F32R, tag="ps1")
                      for s in range(4):
                          nc.tensor.transpose(
                              pst[:, s, :], vt[:, s * 128:(s + 1) * 128], ident)
                      nc.vector.tensor_copy(
                          v_sb[:, 2 * t:2 * t + 2, :, :, 0:64],
                          pst.rearrange("p (pr ko) (h c) -> p pr ko h c",
                                        ko=2, c=64),
                      )
                      if t == 0:
                          nc.vector.tensor_copy(
                              v0_sb[:, :, :, 0:64],
                              pst.rearrange("p s (h c) -> p s h c", c=64),
                          )

                  return [qk(0, qT_sb), qk(1, kT_sb), vproj]

              def emit_tail(jo, to_ps):
                  # combined AV tile for row-tiled out_proj + sums/recip
                  t2 = t_pool.tile([128, 512], F32R, tag="tout")
                  recips = []
                  for h in range(HPC):
                      nc.vector.tensor_copy(
                          t2[64 * h:64 * h + 64, :], to_ps[h][0:64, :])
                      s_row = small_pool.tile([1, 512], F32R, tag="srow")
                      nc.vector.tensor_copy(s_row, to_ps[h][64:65, :])
                      stage = small_pool.tile([4, 128], F32R, tag="stage")
                      for s in range(4):
                          nc.sync.dma_start(
                              out=stage[s:s + 1, :],
                              in_=s_row[:, s * 128:(s + 1) * 128],
                          )
                      pss = ps1.tile([128, 4], F32R, tag="ps1")
                      nc.tensor.transpose(pss, stage, ident[0:4, 0:4])
                      sums_c = small_pool.tile([128, 4], F32, tag="sums")
                      nc.vector.tensor_copy(sums_c, pss)
                      r_c = small_pool.tile([128, 4], F32, tag="recip")
                      nc.vector.reciprocal_approx_fast(out=r_c, in_=sums_c)
                      recips.append(r_c)
                  for c in range(4):
                      i = 4 * jo + c
                      cs = slice(c * 128, (c + 1) * 128)
                      po = []
                      for h in range(HPC):
                          pso = ps1.tile([128, 512], F32, tag="ps1")
                          nc.tensor.matmul(
                              pso,
                              t2[64 * h:64 * h + 64, cs],
                              wo_sb[64 * h:64 * h + 64, :],
                              start=True, stop=True,
                          )
                          po.append(pso)
                      fin = fin_pool.tile([128, 512], F32, tag="fin")
                      nc.vector.tensor_scalar(
                          out=fin, in0=po[0],
                          scalar1=recips[0][:, c:c + 1], scalar2=None,
                          op0=mybir.AluOpType.mult,
                      )
                      nc.vector.scalar_tensor_tensor(
                          out=fin, in0=po[1],
                          scalar=recips[1][:, c:c + 1], in1=fin,
                          op0=mybir.AluOpType.mult, op1=mybir.AluOpType.add,
                      )
                      nc.sync.dma_start(
                          out=out_part[i * 128:(i + 1) * 128, :], in_=fin
                      )

              x_next = emit_xdma(0)
              prev_out = None
              for t in range(NT):
                  j = t
                  qs = slice(j * 512, (j + 1) * 512)
                  if t == 0:
                      for piece in proj_pieces(0, x_next):
                          piece()
                  if t + 1 < NT:
                      x_next = emit_xdma(t + 1)
                      pieces = proj_pieces(t + 1, x_next)
                  else:
                      pieces = []

                  # filler work to interleave after av(kc): tail of j-1
                  # first, then proj pieces of t+1
                  fillers = []
                  if prev_out is not None:
                      po = prev_out
                      fillers.append(lambda po=po: emit_tail(*po))
                      prev_out = None
                  fillers.extend(pieces)

                  to_ps = [psav.tile([65, 512], F32, tag="psav",
                                     name=f"to_ps{_h}") for _h in range(HPC)]
                  nkc = 4 * (j + 1)

                  npair = nkc // 2

                  def emit_av(pair, et):
                      if j == 0:
                          for ko in range(2):
                              kc = 2 * pair + ko
                              for h in range(HPC):
                                  nc.tensor.matmul(
                                      to_ps[h],
                                      v0_sb[:, kc, h, :],
                                      et[:, ko, h, :],
                                      start=(kc == 0), stop=(kc == nkc - 1),
                                  )
                          return
                      for h in range(HPC):
                          nc.tensor.matmul(
                              to_ps[h],
                              v_sb[:, pair, :, h, 0:65],
                              et[:, :, h, :],
                              start=(pair == 0), stop=(pair == npair - 1),
                              perf_mode=mybir.MatmulPerfMode.DoubleRow,
                          )

                  av_q = []
                  for kc in range(nkc):
                      ks = slice(kc * 128, (kc + 1) * 128)
                      sc = ps2.tile([128, 2, 512], F32, tag="ps2")
                      nc.tensor.matmul(
                          sc[:, 0, :], kT_sb[0:64, ks], qT_sb[0:64, qs],
                          start=True, stop=True,
                      )
                      nc.tensor.matmul(
                          sc[:, 1, :], kT_sb[64:128, ks], qT_sb[64:128, qs],
                          start=True, stop=True,
                      )
                      if kc % 2 == 0:
                          et_pair = exp_pool.tile([128, 2, 2, 512],
                                                  F32R if j == 0 else FP8,
                                                  tag="expp")
                      nc.scalar.activation(
                          et_pair[:, kc % 2, :, :], sc,
                          mybir.ActivationFunctionType.Exp,
                          scale=0.125,
                      )
                      if kc >= 4 * j:  # chunk touches the diagonal
                          nc.gpsimd.affine_select(
                              out=et_pair[:, kc % 2, :, :],
                              in_=et_pair[:, kc % 2, :, :],
                              compare_op=mybir.AluOpType.is_ge,
                              fill=0.0,
                              base=512 * j - 128 * kc,
                              channel_multiplier=-1,
                              pattern=[[0, 2], [1, 512]],
                          )
                      if kc % 2 == 1:
                          av_q.append((kc // 2, et_pair))
                      # defer AV by 1 pair so ACT's next scores are never
                      # queued behind an exp-waiting AV on the in-order PE
                      if len(av_q) > 1:
                          emit_av(*av_q.pop(0))
                          if fillers:
                              fillers.pop(0)()
                  while av_q:
                      emit_av(*av_q.pop(0))
                  while fillers:
                      fillers.pop(0)()
                  prev_out = (j, to_ps)
              emit_tail(*prev_out)
    nc.finalize()
    return nc


def make_in_maps(x, qkv_w, qkv_b, out_w):
    x = np.asarray(x, np.float32)
    qkv_w = np.asarray(qkv_w, np.float32)
    qkv_b = np.asarray(qkv_b, np.float32)
    out_w = np.asarray(out_w, np.float32)
    in_maps = []
    for core in range(NCORES):
        b = core // 4
        h0 = HPC * (core % 4)
        rows = np.r_[h0 * D:(h0 + 1) * D, (h0 + 1) * D:(h0 + 2) * D]
        wq = qkv_w[rows]
        wk = qkv_w[C + rows]
        wv = qkv_w[2 * C + rows]
        in_maps.append({
            "xT": np.ascontiguousarray(x[b].T),
            "wqkvT": np.ascontiguousarray(
                np.concatenate([wq, wk, wv], 0).T),
            "bqkv": np.ascontiguousarray(
                np.stack([qkv_b[rows], qkv_b[C + rows], qkv_b[2 * C + rows]], 1)),
            "woT": np.ascontiguousarray(np.concatenate(
                [out_w[:, h0 * D:(h0 + 1) * D].T,
                 out_w[:, (h0 + 1) * D:(h0 + 2) * D].T], 0)),
        })
    return in_maps


def combine(results, out_b):
    parts = [r["out_part"] for r in results]
    out = np.empty((B, N, C), np.float32)
    for b in range(B):
        out[b] = parts[4 * b] + parts[4 * b + 1] + parts[4 * b + 2] \
            + parts[4 * b + 3] + np.asarray(out_b, np.float32)
    return out


_NC = None


def kernel(x, mask, qkv_w, qkv_b, out_w, out_b, **run_kwargs):
    global _NC
    del mask  # causal tril by construction; applied analytically on device
    if _NC is None:
        _NC = build_nc()
    in_maps = make_in_maps(x, qkv_w, qkv_b, out_w)
    res = run_bass_kernel_spmd(_NC, in_maps, list(range(NCORES)), **run_kwargs)
    out = combine(res.results, out_b)
    kernel.last_results = res
    return out


# revision 4
# speedup vs baseline: 1.1530x; 1.0075x over previous
"""Causal MHA block (qkv_proj -> causal softmax attention -> out_proj)
distributed over 8 NeuronCores. v2: chunk-granular scores/exp pipeline.

Sharding: batch x head. B=2, H=8 -> 16 (b,h) units; core i handles batch
i//4 and heads {2*(i%4), 2*(i%4)+1}.

v2 structure (vs v1): scores for BOTH heads of a k-chunk go into one
2-bank PSUM tile ([128, 2, 512]), one exp per chunk covers both heads,
so two chunks fit in PSUM concurrently -> scores(kc+1) overlaps exp(kc)
and the steady state is ACT-bound. AV rides in PE slack. out_proj pairs
are row-tiled ((0,0)/(64,0)) via a combined [128,512] AV tile and a
[128,512] wo layout. qkv projection of tile t+1 is interleaved into the
attention chunk stream of q-tile t so PE never stalls ACT at tile
boundaries.

Device layouts (host pre-transposes -- pure layout, no FLOPs):
  xT     [512, 4096]  x[b].T
  wqkvT  [512, 384]   qkv weight columns for the 2 heads, c-major
  bqkv   [128, 3]     q/k/v bias per f-row ([h0 d64 | h1 d64])
  woT    [128, 512]   rows 0-63 = out_w cols of h0 (transposed), 64-127 = h1
Output per core: out_part [4096, 512]; host sums 4 partials per batch
and adds out_b. All matmuls float32r.
"""

import numpy as np

import concourse.bass as bass
from concourse import bacc
import concourse.mybir as mybir
import concourse.tile as tile
from concourse.bass_utils import run_bass_kernel_spmd

F32 = mybir.dt.float32
F32R = mybir.dt.float32r
FP8 = mybir.dt.float8e4

B, N, C = 2, 4096, 512
H, D = 8, 64
NCORES = 8
HPC = 2               # heads per core
NT = N // 512         # 8 tiles of 512
CC = C // 128         # 4 contraction chunks for the projections
KC = N // 128         # 32 k-chunks


def build_nc(loop_n=None):
    nc = bacc.Bacc()

    xT = nc.declare_dram_parameter("xT", [C, N], F32R, isOutput=False)
    wqkvT = nc.declare_dram_parameter("wqkvT", [C, 3 * 128], F32R, isOutput=False)
    bqkv = nc.declare_dram_parameter("bqkv", [128, 3], F32, isOutput=False)
    woT = nc.declare_dram_parameter("woT", [128, 512], F32R, isOutput=False)
    out_part = nc.declare_dram_parameter("out_part", [N, C], F32, isOutput=True)

    with tile.TileContext(nc) as tc:
        with (
            tc.tile_pool(name="singles", bufs=1) as singles,
            tc.tile_pool(name="xin", bufs=3) as xin_pool,
            tc.tile_pool(name="vtmp", bufs=2) as vtmp_pool,
            tc.tile_pool(name="expp", bufs=5) as exp_pool,
            tc.tile_pool(name="tout", bufs=2) as t_pool,
            tc.tile_pool(name="small", bufs=8) as small_pool,
            tc.tile_pool(name="fin", bufs=3) as fin_pool,
            tc.tile_pool(name="ps1", bufs=2, space="PSUM") as ps1,
            tc.tile_pool(name="psav", bufs=2, space="PSUM") as psav,
            tc.tile_pool(name="ps2", bufs=2, space="PSUM") as ps2,
        ):
            # ---- persistent tiles ----
            w_sb = singles.tile([128, CC, 3, 128], F32R)       # wqkvT chunks
            wo_sb = singles.tile([128, 512], F32R)             # [h0 | h1] rows
            bqkv_sb = singles.tile([128, 3], F32)
            ident_f = singles.tile([128, 128], F32)
            ident = singles.tile([128, 128], F32R)
            ones_f = singles.tile([128, 64], F32)
            qT_sb = singles.tile([128, N], F32R)               # [h0 d|h1 d] x n
            kT_sb = singles.tile([128, N], F32R)
            v_sb = singles.tile([128, KC // 2, 2, 2, 80], FP8)  # [pair, ko, h, v|1]
            v0_sb = singles.tile([128, 4, 2, 65], F32R)         # chunks 0-3, exact

            nc.gpsimd.dma_start(
                out=w_sb,
                in_=wqkvT[:].rearrange("(cc p) (pr f) -> p cc pr f", p=128, f=128),
            )
            nc.gpsimd.dma_start(out=wo_sb, in_=woT[:])
            nc.gpsimd.dma_start(out=bqkv_sb, in_=bqkv[:])
            from concourse.masks import make_identity

            make_identity(nc, ident_f)
            nc.vector.tensor_copy(ident, ident_f)
            nc.gpsimd.memset(ones_f, 1.0)
            nc.vector.tensor_copy(
                v_sb[:, :, :, :, 64:65],
                ones_f[:, 0:64].rearrange("p (a b c d) -> p a b c d",
                                          b=2, c=2, d=1),
            )
            nc.vector.tensor_copy(
                v0_sb[:, :, :, 64:65],
                ones_f[:, 0:8].rearrange("p (a b c) -> p a b c", b=2, c=1),
            )

            import contextlib
            loop_cm = tc.For_i(0, loop_n, 1) if loop_n else contextlib.nullcontext()
            with loop_cm:
              def emit_xdma(t):
                  ns = slice(t * 512, (t + 1) * 512)
                  x_t = xin_pool.tile([128, CC, 512], F32R, tag="xin")
                  nc.sync.dma_start(
                      out=x_t,
                      in_=xT[:, ns].rearrange("(cc p) n -> p cc n", p=128),
                  )
                  return x_t

              def proj_pieces(t, x_t):
                  """qkv projection of tile t as a list of closures (PE
                  filler pieces interleaved into the attention stream)."""
                  ns = slice(t * 512, (t + 1) * 512)

                  def qk(pr, dst):
                      def f():
                          ps = ps1.tile([128, 512], F32, tag="ps1")
                          for cc in range(CC):
                              nc.tensor.matmul(
                                  ps, w_sb[:, cc, pr, :], x_t[:, cc, :],
                                  start=(cc == 0), stop=(cc == CC - 1),
                              )
                          nc.vector.tensor_scalar(
                              out=dst[:, ns], in0=ps,
                              scalar1=bqkv_sb[:, pr:pr + 1], scalar2=None,
                              op0=mybir.AluOpType.add,
                          )
                      return f

                  def vproj():
                      ps = ps1.tile([128, 512], F32, tag="ps1")
                      for cc in range(CC):
                          nc.tensor.matmul(
                              ps, w_sb[:, cc, 2, :], x_t[:, cc, :],
                              start=(cc == 0), stop=(cc == CC - 1),
                          )
                      vt = vtmp_pool.tile([128, 512], F32R, tag="vtmp")
                      nc.vector.tensor_scalar(
                          out=vt, in0=ps,
                          scalar1=bqkv_sb[:, 2:3], scalar2=None,
                          op0=mybir.AluOpType.add,
                      )
                      pst = ps1.tile([128, 4, 128], F32R, tag="ps1")
                      for s in range(4):
                          nc.tensor.transpose(
                              pst[:, s, :], vt[:, s * 128:(s + 1) * 128], ident)
                      nc.vector.tensor_copy(
                          v_sb[:, 2 * t:2 * t + 2, :, :, 0:64],
                          pst.rearrange("p (pr ko) (h c) -> p pr ko h c",
                                        ko=2, c=64),
                      )
                      if t == 0:
                          nc.vector.tensor_copy(
                              v0_sb[:, :, :, 0:64],
                              pst.rearrange("p s (h c) -> p s h c", c=64),
                          )

                  return [qk(0, qT_sb), qk(1, kT_sb), vproj]

              def emit_tail(jo, to_ps):
                  # combined AV tile for row-tiled out_proj + sums/recip
                  t2 = t_pool.tile([128, 512], F32R, tag="tout")
                  recips = []
                  for h in range(HPC):
                      nc.vector.tensor_copy(
                          t2[64 * h:64 * h + 64, :], to_ps[h][0:64, :])
                      s_row = small_pool.tile([1, 512], F32R, tag="srow")
                      nc.vector.tensor_copy(s_row, to_ps[h][64:65, :])
                      stage = small_pool.tile([4, 128], F32R, tag="stage")
                      for s in range(4):
                          nc.sync.dma_start(
                              out=stage[s:s + 1, :],
                              in_=s_row[:, s * 128:(s + 1) * 128],
                          )
                      pss = ps1.tile([128, 4], F32R, tag="ps1")
                      nc.tensor.transpose(pss, stage, ident[0:4, 0:4])
                      sums_c = small_pool.tile([128, 4], F32, tag="sums")
                      nc.vector.tensor_copy(sums_c, pss)
                      r_c = small_pool.tile([128, 4], F32, tag="recip")
                      nc.vector.reciprocal_approx_fast(out=r_c, in_=sums_c)
                      recips.append(r_c)
                  for c in range(4):
                      i = 4 * jo + c
                      cs = slice(c * 128, (c + 1) * 128)
                      po = []
                      for h in range(HPC):
                          pso = ps1.tile([128, 512], F32, tag="ps1")
                          nc.tensor.matmul(
                              pso,
                              t2[64 * h:64 * h + 64, cs],
                              wo_sb[64 * h:64 * h + 64, :],
                              start=True, stop=True,
                          )
                          po.append(pso)
                      fin = fin_pool.tile([128, 512], F32, tag="fin")
                      nc.vector.tensor_scalar(
                          out=fin, in0=po[0],
                          scalar1=recips[0][:, c:c + 1], scalar2=None,
                          op0=mybir.AluOpType.mult,
                      )
                      nc.vector.scalar_tensor_tensor(
                          out=fin, in0=po[1],
                          scalar=recips[1][:, c:c + 1], in1=fin,
                          op0=mybir.AluOpType.mult, op1=mybir.AluOpType.add,
                      )
                      nc.sync.dma_start(
                          out=out_part[i * 128:(i + 1) * 128, :], in_=fin
                      )

              x_next = emit_xdma(0)
              prev_out = None
              for t in range(NT):
                  j = t
                  qs = slice(j * 512, (j + 1) * 512)
                  if t == 0:
                      for piece in proj_pieces(0, x_next):
                          piece()
                  if t + 1 < NT:
                      x_next = emit_xdma(t + 1)
                      pieces = proj_pieces(t + 1, x_next)
                  else:
                      pieces = []

                  # filler work to interleave after av(kc): tail of j-1
                  # first, then proj pieces of t+1
                  fillers = []
                  if prev_out is not None:
                      po = prev_out
                      fillers.append(lambda po=po: emit_tail(*po))
                      prev_out = None
                  fillers.extend(pieces)

                  to_ps = [psav.tile([65, 512], F32, tag="psav",
                                     name=f"to_ps{_h}") for _h in range(HPC)]
                  nkc = 4 * (j + 1)

                  npair = nkc // 2

                  def emit_av(pair, et):
                      if j == 0:
                          for ko in range(2):
                              kc = 2 * pair + ko
                              for h in range(HPC):
                                  nc.tensor.matmul(
                                      to_ps[h],
                                      v0_sb[:, kc, h, :],
                                      et[:, ko, h, :],
                                      start=(kc == 0), stop=(kc == nkc - 1),
                                  )
                          return
                      for h in range(HPC):
                          nc.tensor.matmul(
                              to_ps[h],
                              v_sb[:, pair, :, h, 0:65],
                              et[:, :, h, :],
                              start=(pair == 0), stop=(pair == npair - 1),
                              perf_mode=mybir.MatmulPerfMode.DoubleRow,
                          )

                  av_q = []
                  for kc in range(nkc):
                      ks = slice(kc * 128, (kc + 1) * 128)
                      sc = ps2.tile([128, 2, 512], F32, tag="ps2")
                      nc.tensor.matmul(
                          sc[:, 0, :], kT_sb[0:64, ks], qT_sb[0:64, qs],
                          start=True, stop=True,
                      )
                      nc.tensor.matmul(
                          sc[:, 1, :], kT_sb[64:128, ks], qT_sb[64:128, qs],
                          start=True, stop=True,
                      )
                      if kc % 2 == 0:
                          et_pair = exp_pool.tile([128, 2, 2, 512],
                                                  F32R if j == 0 else FP8,
                                                  tag="expp")
                      nc.scalar.activation(
                          et_pair[:, kc % 2, :, :], sc,
                          mybir.ActivationFunctionType.Exp,
                          scale=0.125,
                      )
                      if kc >= 4 * j:  # chunk touches the diagonal
                          nc.gpsimd.affine_select(
                              out=et_pair[:, kc % 2, :, :],
                              in_=et_pair[:, kc % 2, :, :],
                              compare_op=mybir.AluOpType.is_ge,
                              fill=0.0,
                              base=512 * j - 128 * kc,
                              channel_multiplier=-1,
                              pattern=[[0, 2], [1, 512]],
                          )
                      if kc % 2 == 1:
                          av_q.append((kc // 2, et_pair))
                      # defer AV by 1 pair so ACT's next scores are never
                      # queued behind an exp-waiting AV on the in-order PE
                      if len(av_q) > 2:
                          emit_av(*av_q.pop(0))
                          if fillers:
                              fillers.pop(0)()
                      elif fillers and kc % 2 == 1:
                          fillers.pop(0)()
                  while av_q:
                      emit_av(*av_q.pop(0))
                  while fillers:
                      fillers.pop(0)()
                  prev_out = (j, to_ps)
              emit_tail(*prev_out)
    nc.finalize()
    return nc


def make_in_maps(x, qkv_w, qkv_b, out_w):
    x = np.asarray(x, np.float32)
    qkv_w = np.asarray(qkv_w, np.float32)
    qkv_b = np.asarray(qkv_b, np.float32)
    out_w = np.asarray(out_w, np.float32)
    in_maps = []
    for core in range(NCORES):
        b = core // 4
        h0 = HPC * (core % 4)
        rows = np.r_[h0 * D:(h0 + 1) * D, (h0 + 1) * D:(h0 + 2) * D]
        wq = qkv_w[rows]
        wk = qkv_w[C + rows]
        wv = qkv_w[2 * C + rows]
        in_maps.append({
            "xT": np.ascontiguousarray(x[b].T),
            "wqkvT": np.ascontiguousarray(
                np.concatenate([wq, wk, wv], 0).T),
            "bqkv": np.ascontiguousarray(
                np.stack([qkv_b[rows], qkv_b[C + rows], qkv_b[2 * C + rows]], 1)),
            "woT": np.ascontiguousarray(np.concatenate(
                [out_w[:, h0 * D:(h0 + 1) * D].T,
                 out_w[:, (h0 + 1) * D:(h0 + 2) * D].T], 0)),
        })
    return in_maps


def combine(results, out_b):
    parts = [r["out_part"] for r in results]
    out = np.empty((B, N, C), np.float32)
    for b in range(B):
        out[b] = parts[4 * b] + parts[4 * b + 1] + parts[4 * b + 2] \
            + parts[4 * b + 3] + np.asarray(out_b, np.float32)
    return out


_NC = None


def kernel(x, mask, qkv_w, qkv_b, out_w, out_b, **run_kwargs):
    global _NC
    del mask  # causal tril by construction; applied analytically on device
    if _NC is None:
        _NC = build_nc()
    in_maps = make_in_maps(x, qkv_w, qkv_b, out_w)
    res = run_bass_kernel_spmd(_NC, in_maps, list(range(NCORES)), **run_kwargs)
    out = combine(res.results, out_b)
    kernel.last_results = res
    return out


# revision 7
# speedup vs baseline: 1.1803x; 1.0237x over previous
"""Causal MHA block (qkv_proj -> causal softmax attention -> out_proj)
distributed over 8 NeuronCores. v2: chunk-granular scores/exp pipeline.

Sharding: batch x head. B=2, H=8 -> 16 (b,h) units; core i handles batch
i//4 and heads {2*(i%4), 2*(i%4)+1}.

v2 structure (vs v1): scores for BOTH heads of a k-chunk go into one
2-bank PSUM tile ([128, 2, 512]), one exp per chunk covers both heads,
so two chunks fit in PSUM concurrently -> scores(kc+1) overlaps exp(kc)
and the steady state is ACT-bound. AV rides in PE slack. out_proj pairs
are row-tiled ((0,0)/(64,0)) via a combined [128,512] AV tile and a
[128,512] wo layout. qkv projection of tile t+1 is interleaved into the
attention chunk stream of q-tile t so PE never stalls ACT at tile
boundaries.

Device layouts (host pre-transposes -- pure layout, no FLOPs):
  xT     [512, 4096]  x[b].T
  wqkvT  [512, 384]   qkv weight columns for the 2 heads, c-major
  bqkv   [128, 3]     q/k/v bias per f-row ([h0 d64 | h1 d64])
  woT    [128, 512]   rows 0-63 = out_w cols of h0 (transposed), 64-127 = h1
Output per core: out_part [4096, 512]; host sums 4 partials per batch
and adds out_b. All matmuls float32r.
"""

import numpy as np

import concourse.bass as bass
from concourse import bacc
import concourse.mybir as mybir
import concourse.tile as tile
from concourse.bass_utils import run_bass_kernel_spmd

F32 = mybir.dt.float32
F32R = mybir.dt.float32r
FP8 = mybir.dt.float8e4

B, N, C = 2, 4096, 512
H, D = 8, 64
NCORES = 8
HPC = 2               # heads per core
NT = N // 512         # 8 tiles of 512
CC = C // 128         # 4 contraction chunks for the projections
KC = N // 128         # 32 k-chunks


def build_nc(loop_n=None):
    nc = bacc.Bacc()

    xT = nc.declare_dram_parameter("xT", [C, N], F32R, isOutput=False)
    wqkvT = nc.declare_dram_parameter("wqkvT", [C, 3 * 128], F32R, isOutput=False)
    bqkv = nc.declare_dram_parameter("bqkv", [128, 3], F32, isOutput=False)
    woT = nc.declare_dram_parameter("woT", [128, 512], F32R, isOutput=False)
    out_part = nc.declare_dram_parameter("out_part", [N, C], F32, isOutput=True)

    with tile.TileContext(nc) as tc:
        with (
            tc.tile_pool(name="singles", bufs=1) as singles,
            tc.tile_pool(name="xin", bufs=3) as xin_pool,
            tc.tile_pool(name="vtmp", bufs=2) as vtmp_pool,
            tc.tile_pool(name="expp", bufs=6) as exp_pool,
            tc.tile_pool(name="tout", bufs=2) as t_pool,
            tc.tile_pool(name="small", bufs=8) as small_pool,
            tc.tile_pool(name="fin", bufs=3) as fin_pool,
            tc.tile_pool(name="ps1", bufs=2, space="PSUM") as ps1,
            tc.tile_pool(name="psav", bufs=2, space="PSUM") as psav,
            tc.tile_pool(name="ps2", bufs=2, space="PSUM") as ps2,
        ):
            # ---- persistent tiles ----
            w_sb = singles.tile([128, CC, 3, 128], F32R)       # wqkvT chunks
            wo_sb = singles.tile([128, 512], F32R)             # [h0 | h1] rows
            bqkv_sb = singles.tile([128, 3], F32)
            ident_f = singles.tile([128, 128], F32)
            ident = singles.tile([128, 128], F32R)
            ones_f = singles.tile([128, 64], F32)
            qT_sb = singles.tile([128, N], F32R)               # [h0 d|h1 d] x n
            kT_sb = singles.tile([128, N], F32R)
            v_sb = singles.tile([128, KC // 2, 2, 2, 80], FP8)  # [pair, ko, h, v|1]
            v0_sb = singles.tile([128, 4, 2, 65], F32R)         # chunks 0-3, exact

            nc.gpsimd.dma_start(
                out=w_sb,
                in_=wqkvT[:].rearrange("(cc p) (pr f) -> p cc pr f", p=128, f=128),
            )
            nc.gpsimd.dma_start(out=wo_sb, in_=woT[:])
            nc.gpsimd.dma_start(out=bqkv_sb, in_=bqkv[:])
            from concourse.masks import make_identity

            make_identity(nc, ident_f)
            nc.vector.tensor_copy(ident, ident_f)
            nc.gpsimd.memset(ones_f, 1.0)
            nc.vector.tensor_copy(
                v_sb[:, :, :, :, 64:65],
                ones_f[:, 0:64].rearrange("p (a b c d) -> p a b c d",
                                          b=2, c=2, d=1),
            )
            nc.vector.tensor_copy(
                v0_sb[:, :, :, 64:65],
                ones_f[:, 0:8].rearrange("p (a b c) -> p a b c", b=2, c=1),
            )

            import contextlib
            loop_cm = tc.For_i(0, loop_n, 1) if loop_n else contextlib.nullcontext()
            with loop_cm:
              def emit_xdma(t):
                  ns = slice(t * 512, (t + 1) * 512)
                  x_t = xin_pool.tile([128, CC, 512], F32R, tag="xin")
                  nc.sync.dma_start(
                      out=x_t,
                      in_=xT[:, ns].rearrange("(cc p) n -> p cc n", p=128),
                  )
                  return x_t

              def proj_pieces(t, x_t):
                  """qkv projection of tile t as a list of closures (PE
                  filler pieces interleaved into the attention stream)."""
                  ns = slice(t * 512, (t + 1) * 512)

                  def qk(pr, dst):
                      def f():
                          ps = ps1.tile([128, 512], F32, tag="ps1")
                          for cc in range(CC):
                              nc.tensor.matmul(
                                  ps, w_sb[:, cc, pr, :], x_t[:, cc, :],
                                  start=(cc == 0), stop=(cc == CC - 1),
                              )
                          nc.vector.tensor_scalar(
                              out=dst[:, ns], in0=ps,
                              scalar1=bqkv_sb[:, pr:pr + 1], scalar2=None,
                              op0=mybir.AluOpType.add,
                          )
                      return f

                  def vproj():
                      ps = ps1.tile([128, 512], F32, tag="ps1")
                      for cc in range(CC):
                          nc.tensor.matmul(
                              ps, w_sb[:, cc, 2, :], x_t[:, cc, :],
                              start=(cc == 0), stop=(cc == CC - 1),
                          )
                      vt = vtmp_pool.tile([128, 512], F32R, tag="vtmp")
                      nc.vector.tensor_scalar(
                          out=vt, in0=ps,
                          scalar1=bqkv_sb[:, 2:3], scalar2=None,
                          op0=mybir.AluOpType.add,
                      )
                      pst = ps1.tile([128, 4, 128], F32R, tag="ps1")
                      for s in range(4):
                          nc.tensor.transpose(
                              pst[:, s, :], vt[:, s * 128:(s + 1) * 128], ident)
                      nc.vector.tensor_copy(
                          v_sb[:, 2 * t:2 * t + 2, :, :, 0:64],
                          pst.rearrange("p (pr ko) (h c) -> p pr ko h c",
                                        ko=2, c=64),
                      )
                      if t == 0:
                          nc.vector.tensor_copy(
                              v0_sb[:, :, :, 0:64],
                              pst.rearrange("p s (h c) -> p s h c", c=64),
                          )

                  return [qk(0, qT_sb), qk(1, kT_sb), vproj]

              def emit_tail(jo, to_ps):
                  # combined AV tile for row-tiled out_proj + sums/recip
                  t2 = t_pool.tile([128, 512], F32R, tag="tout")
                  recips = []
                  for h in range(HPC):
                      nc.vector.tensor_copy(
                          t2[64 * h:64 * h + 64, :], to_ps[h][0:64, :])
                      s_row = small_pool.tile([1, 512], F32R, tag="srow")
                      nc.vector.tensor_copy(s_row, to_ps[h][64:65, :])
                      stage = small_pool.tile([4, 128], F32R, tag="stage")
                      nc.sync.dma_start(
                          out=stage,
                          in_=s_row.rearrange("a (s f) -> a s f", s=4),
                      )
                      pss = ps1.tile([128, 4], F32R, tag="ps1")
                      nc.tensor.transpose(pss, stage, ident[0:4, 0:4])
                      sums_c = small_pool.tile([128, 4], F32, tag="sums")
                      nc.vector.tensor_copy(sums_c, pss)
                      r_c = small_pool.tile([128, 4], F32, tag="recip")
                      nc.vector.reciprocal_approx_fast(out=r_c, in_=sums_c)
                      recips.append(r_c)
                  for c in range(4):
                      i = 4 * jo + c
                      cs = slice(c * 128, (c + 1) * 128)
                      po = []
                      for h in range(HPC):
                          pso = ps1.tile([128, 512], F32, tag="ps1")
                          nc.tensor.matmul(
                              pso,
                              t2[64 * h:64 * h + 64, cs],
                              wo_sb[64 * h:64 * h + 64, :],
                              start=True, stop=True,
                          )
                          po.append(pso)
                      fin = fin_pool.tile([128, 512], F32, tag="fin")
                      nc.vector.tensor_scalar(
                          out=fin, in0=po[0],
                          scalar1=recips[0][:, c:c + 1], scalar2=None,
                          op0=mybir.AluOpType.mult,
                      )
                      nc.vector.scalar_tensor_tensor(
                          out=fin, in0=po[1],
                          scalar=recips[1][:, c:c + 1], in1=fin,
                          op0=mybir.AluOpType.mult, op1=mybir.AluOpType.add,
                      )
                      nc.sync.dma_start(
                          out=out_part[i * 128:(i + 1) * 128, :], in_=fin
                      )

              x_next = emit_xdma(0)
              prev_out = None
              for t in range(NT):
                  j = t
                  qs = slice(j * 512, (j + 1) * 512)
                  if t == 0:
                      for piece in proj_pieces(0, x_next):
                          piece()
                  if t + 1 < NT:
                      x_next = emit_xdma(t + 1)
                      pieces = proj_pieces(t + 1, x_next)
                  else:
                      pieces = []

                  # filler work to interleave after av(kc): tail of j-1
                  # first, then proj pieces of t+1
                  fillers = []
                  if prev_out is not None:
                      po = prev_out
                      fillers.append(lambda po=po: emit_tail(*po))
                      prev_out = None
                  fillers.extend(pieces)

                  to_ps = [psav.tile([65, 512], F32, tag="psav",
                                     name=f"to_ps{_h}") for _h in range(HPC)]
                  nkc = 4 * (j + 1)

                  npair = nkc // 2

                  def emit_av(pair, et):
                      # last pair of each q-tile covers chunks (4j+2, 4j+3)
                      # whose live queries are >= 256; skip the dead half
                      a0 = 256 if pair == npair - 1 else 0
                      if j == 0:
                          for ko in range(2):
                              kc = 2 * pair + ko
                              for h in range(HPC):
                                  nc.tensor.matmul(
                                      to_ps[h][:, a0:512],
                                      v0_sb[:, kc, h, :],
                                      et[:, ko, h, a0:512],
                                      start=(kc == 0), stop=(kc == nkc - 1),
                                  )
                          return
                      for h in range(HPC):
                          nc.tensor.matmul(
                              to_ps[h][:, a0:512],
                              v_sb[:, pair, :, h, 0:65],
                              et[:, :, h, a0:512],
                              start=(pair == 0), stop=(pair == npair - 1),
                              perf_mode=mybir.MatmulPerfMode.DoubleRow,
                          )

                  av_q = []
                  for kc in range(nkc):
                      ks = slice(kc * 128, (kc + 1) * 128)
                      # diagonal chunks: queries < 128*(kc-4j) are fully
                      # masked; skip them in scores (min width 256 to avoid
                      # the fp32r small-ap penalty) and in exp (the affine
                      # mask zeroes the dead prefix of et anyway)
                      q0 = 128 * (kc - 4 * j) if kc >= 4 * j else 0
                      sq0 = min(q0, 256)
                      sqs = slice(j * 512 + sq0, (j + 1) * 512)
                      sc = ps2.tile([128, 2, 512], F32, tag="ps2")
                      nc.tensor.matmul(
                          sc[:, 0, sq0:512], kT_sb[0:64, ks],
                          qT_sb[0:64, sqs],
                          start=True, stop=True,
                      )
                      nc.tensor.matmul(
                          sc[:, 1, sq0:512], kT_sb[64:128, ks],
                          qT_sb[64:128, sqs],
                          start=True, stop=True,
                      )
                      if kc % 2 == 0:
                          et_pair = exp_pool.tile([128, 2, 2, 512],
                                                  F32R if j == 0 else FP8,
                                                  tag="expp")
                      nc.scalar.activation(
                          et_pair[:, kc % 2, :, q0:512], sc[:, :, q0:512],
                          mybir.ActivationFunctionType.Exp,
                          scale=0.125,
                      )
                      if kc >= 4 * j:  # chunk touches the diagonal
                          nc.gpsimd.affine_select(
                              out=et_pair[:, kc % 2, :, :],
                              in_=et_pair[:, kc % 2, :, :],
                              compare_op=mybir.AluOpType.is_ge,
                              fill=0.0,
                              base=512 * j - 128 * kc,
                              channel_multiplier=-1,
                              pattern=[[0, 2], [1, 512]],
                          )
                      if kc % 2 == 1:
                          av_q.append((kc // 2, et_pair))
                      # defer AV by 1 pair so ACT's next scores are never
                      # queued behind an exp-waiting AV on the in-order PE
                      if len(av_q) > 2:
                          emit_av(*av_q.pop(0))
                          if fillers:
                              fillers.pop(0)()
                      elif fillers and kc % 2 == 1:
                          fillers.pop(0)()
                  while av_q:
                      emit_av(*av_q.pop(0))
                  while fillers:
                      fillers.pop(0)()
                  prev_out = (j, to_ps)
              emit_tail(*prev_out)
    nc.finalize()
    return nc


def make_in_maps(x, qkv_w, qkv_b, out_w):
    x = np.asarray(x, np.float32)
    qkv_w = np.asarray(qkv_w, np.float32)
    qkv_b = np.asarray(qkv_b, np.float32)
    out_w = np.asarray(out_w, np.float32)
    in_maps = []
    for core in range(NCORES):
        b = core // 4
        h0 = HPC * (core % 4)
        rows = np.r_[h0 * D:(h0 + 1) * D, (h0 + 1) * D:(h0 + 2) * D]
        wq = qkv_w[rows]
        wk = qkv_w[C + rows]
        wv = qkv_w[2 * C + rows]
        in_maps.append({
            "xT": np.ascontiguousarray(x[b].T),
            "wqkvT": np.ascontiguousarray(
                np.concatenate([wq, wk, wv], 0).T),
            "bqkv": np.ascontiguousarray(
                np.stack([qkv_b[rows], qkv_b[C + rows], qkv_b[2 * C + rows]], 1)),
            "woT": np.ascontiguousarray(np.concatenate(
                [out_w[:, h0 * D:(h0 + 1) * D].T,
                 out_w[:, (h0 + 1) * D:(h0 + 2) * D].T], 0)),
        })
    return in_maps


def combine(results, out_b):
    parts = [r["out_part"] for r in results]
    out = np.empty((B, N, C), np.float32)
    for b in range(B):
        out[b] = parts[4 * b] + parts[4 * b + 1] + parts[4 * b + 2] \
            + parts[4 * b + 3] + np.asarray(out_b, np.float32)
    return out


_NC = None


def kernel(x, mask, qkv_w, qkv_b, out_w, out_b, **run_kwargs):
    global _NC
    del mask  # causal tril by construction; applied analytically on device
    if _NC is None:
        _NC = build_nc()
    in_maps = make_in_maps(x, qkv_w, qkv_b, out_w)
    res = run_bass_kernel_spmd(_NC, in_maps, list(range(NCORES)), **run_kwargs)
    out = combine(res.results, out_b)
    kernel.last_results = res
    return out
